# revision 23
# baseline (speedup 1.0000x reference)
"""ColAttention TRN2 kernel: out = gamma * colattn(x) + x.

Sharding: width. Core k gets x[:, :, :, 16k:16(k+1)]. Per core: 8 batches x 16
width columns = 128 independent attention problems over h=128.

v7 design (v2 baseline 380us, v3 288us, v6 250us):
  - v3 structure: all-bf16 QK/scores, fp8 DoubleRow V, per-column softmax
    (ACT exp + accum_out, DVE recip + normalize), PE transpose of attn,
    residual in the DVE drain, w-major slab + host-side transposes
  - v6 fixes from the v3/v5 measurements:
      * consts go on the scalar HWDGE ring so the sync ring starts the first
        batch's slabs immediately (v3 had a 15us startup stall)
      * bf16 output slab: halves the DVE drain write cost and the HBM store
        traffic; host upcasts to f32 (rel-err budget allows the 0.3%)
      * v5's gpsimd-ring stores and delayed-priority drains are reverted --
        together they corrupted results (absmax 0.15 -> 2.18) and cost 11us
  - v7/v8: half-slab early stores to shrink the end-of-kernel tail,
    single-DMA slab loads, fp8 slab loaded first (warms PE sooner). Tried
    and reverted: paired drains (PSUM bank limit), normalize on gpsimd
    (gpsimd tensor_scalar is 2060ns vs 330 on DVE -- 6x slower)
"""

import numpy as np
import ml_dtypes

import concourse.bass as bass
from concourse import bacc, mybir
from concourse.tile import TileContext
from concourse.bass_utils import run_bass_kernel_spmd

f32 = mybir.dt.float32
bf16 = mybir.dt.bfloat16
f8 = mybir.dt.float8e4
AF = mybir.ActivationFunctionType
ALU = mybir.AluOpType
DR = mybir.MatmulPerfMode.DoubleRow
FP8_WSCALE = 32.0

N_CORES = 8
B, C, H, W = 8, 512, 128, 128
WT = W // N_CORES          # 16 w-columns per core
DQ = 64
NCH = C // 128             # 4 c-chunks

TRACE = False              # set True from test.py for profiling
LAST_RESULTS = None


def _build(bv_is_zero: bool, bqk_is_zero: bool):
    nc = bacc.Bacc("TRN2", num_devices=N_CORES, debug=False)

    # w-major bf16 slab input: (B, C, WT, H)
    xb_d = nc.dram_tensor("xb", (B, C, WT, H), bf16, kind="ExternalInput")
    # fp8 slab for V proj, baseline layout: (B, C, H, WT)
    x8_d = nc.dram_tensor("x8", (B, C, H, WT), f8, kind="ExternalInput")
    wqk_d = nc.dram_tensor("wqkT", (C, 128), bf16, kind="ExternalInput")
    bqk_d = nc.dram_tensor("bqk", (128, 1), f32, kind="ExternalInput")
    wv_d = nc.dram_tensor("wvT", (C, C), f8, kind="ExternalInput")
    gbv_d = nc.dram_tensor("gbv", (128, NCH), f32, kind="ExternalInput")
    out_d = nc.dram_tensor("out", (B, C, WT, H), bf16, kind="ExternalOutput")
    id_d = nc.inline_tensor(np.eye(128, dtype=ml_dtypes.bfloat16), name="id128")

    xba = xb_d.ap()
    x8a = x8_d.ap()
    oa = out_d.ap()

    with TileContext(nc) as tc:
        with (
            tc.tile_pool(name="const", bufs=1) as cpool,
            tc.tile_pool(name="xs", bufs=2) as xspool,
            tc.tile_pool(name="x8", bufs=2) as x8pool,
            tc.tile_pool(name="os", bufs=2) as ospool,
            tc.tile_pool(name="qk", bufs=2) as qkpool,
            tc.tile_pool(name="small", bufs=3) as spool,
            tc.tile_pool(name="pqk", bufs=1, space="PSUM") as pqk,
            tc.tile_pool(name="pvt", bufs=2, space="PSUM") as pvt,
            tc.tile_pool(name="psc", bufs=2, space="PSUM") as psc,
            tc.tile_pool(name="ptp", bufs=1, space="PSUM") as ptp,
            tc.tile_pool(name="pav", bufs=2, space="PSUM") as pav,
        ):
            # ---- constants: scalar HWDGE ring, so the sync ring starts on
            # the first batch's slabs immediately ----
            wqk_sb = cpool.tile([128, 128 * NCH], bf16, name="wqk_sb")
            for ci in range(NCH):
                nc.scalar.dma_start(wqk_sb[:, ci * 128:(ci + 1) * 128],
                                    wqk_d.ap()[ci * 128:(ci + 1) * 128, :])
            wv_sb = cpool.tile([128, 512 * NCH], f8, name="wv_sb")
            for ci in range(NCH):
                nc.scalar.dma_start(wv_sb[:, ci * 512:(ci + 1) * 512],
                                    wv_d.ap()[ci * 128:(ci + 1) * 128, :])
            wv84 = wv_sb[:].rearrange("p (c n) -> p c n", c=NCH)
            bqk_sb = cpool.tile([128, 1], f32, name="bqk_sb")
            nc.scalar.dma_start(bqk_sb[:], bqk_d.ap())
            gbv_sb = cpool.tile([128, NCH], f32, name="gbv_sb")
            nc.scalar.dma_start(gbv_sb[:], gbv_d.ap())
            id_sb = cpool.tile([128, 128], bf16, name="id_sb")
            nc.scalar.dma_start(id_sb[:], id_d.ap())

            for b in range(B):
                # ---- batch prologue, hoisted into the previous batch's
                # w-loop. DMAs go much earlier than the QK matmuls: a QK
                # matmul entering the in-order PE queue before its slab DMA
                # lands head-of-line-blocks the current batch's matmuls ----
                with tc.high_priority(offset=0 if b == 0 else 300):
                    # bf16 slab, w-major: (p, ci, w, h)
                    xs = xspool.tile([128, NCH * WT * H], bf16, tag="xs",
                                     name=f"xs{b}")
                    xs4 = xs[:].rearrange("p (c w h) -> p c w h", c=NCH, w=WT)
                    # fp8 slab, (p, ci, h, w) baseline layout for DoubleRow
                    xf8 = x8pool.tile([128, NCH * H * WT], f8, tag="x8",
                                      name=f"x8{b}")
                    xf84 = xf8[:].rearrange("p (c h w) -> p c h w", c=NCH, w=WT)
                    xbv = xba[b].rearrange("(c p) w h -> p c w h", c=NCH)
                    if b == 0:
                        # split the first load so the first QK matmul group
                        # (which reads the first 512 pixels of every chunk)
                        # only waits on 1MB; fp8 slab lands before half B so
                        # the first V matmuls aren't the last thing waiting
                        nc.sync.dma_start(xs4[:, :, 0:WT // 2, :],
                                          xbv[:, :, 0:WT // 2, :])
                        nc.sync.dma_start(
                            xf84[:],
                            x8a[b].rearrange("(c p) h w -> p c h w", c=NCH))
                        nc.sync.dma_start(xs4[:, :, WT // 2:WT, :],
                                          xbv[:, :, WT // 2:WT, :])
                    else:
                        nc.sync.dma_start(xs4[:], xbv)
                        nc.sync.dma_start(
                            xf84[:],
                            x8a[b].rearrange("(c p) h w -> p c h w", c=NCH))

                with tc.high_priority(offset=0 if b == 0 else 120):
                    # QK projection: bf16, n-tiles of 512 over (w, h)
                    qk_sb = qkpool.tile([128, WT * H], bf16, tag="qk",
                                        name=f"qk{b}")
                    ks = qkpool.tile([64, WT * H], bf16, tag="ks", name=f"ks{b}")
                    for nt in range(WT * H // 512):
                        qkp = pqk.tile([128, 512], f32, tag="qkp")
                        for ci in range(NCH):
                            nc.tensor.matmul(
                                qkp[:],
                                wqk_sb[:, ci * 128:(ci + 1) * 128],
                                xs[:, ci * 2048 + nt * 512:
                                   ci * 2048 + (nt + 1) * 512],
                                start=(ci == 0), stop=(ci == NCH - 1))
                        dst = qk_sb[:, nt * 512:(nt + 1) * 512]
                        if bqk_is_zero:
                            if nt % 2 == 0:
                                nc.scalar.activation(dst, qkp[:], AF.Copy)
                            else:
                                nc.vector.tensor_copy(dst, qkp[:])
                        else:
                            if nt % 2 == 0:
                                nc.scalar.activation(dst, qkp[:], AF.Identity,
                                                     bias=bqk_sb[:])
                            else:
                                nc.vector.tensor_scalar_add(dst, qkp[:],
                                                            bqk_sb[:])
                        # K rows 64:128 -> partitions 0:63 (scores needs
                        # matching base partitions)
                        nc.sync.dma_start(ks[:, nt * 512:(nt + 1) * 512],
                                          qk_sb[64:128, nt * 512:(nt + 1) * 512])
                qk3 = qk_sb[:].rearrange("p (w h) -> p w h", w=WT)
                ks3 = ks[:].rearrange("p (w h) -> p w h", w=WT)
                osb = ospool.tile([128, NCH * WT * H], bf16, tag="os",
                                  name=f"os{b}")
                os4 = osb[:].rearrange("p (c w h) -> p c w h", c=NCH, w=WT)

                # ---- one-column software pipeline: column w's scores +
                # V-proj (chain-independent PE work) and softmax head are
                # emitted BEFORE column w-1's transpose/AV/drain, so the
                # transpose matmul no longer enters the in-order PE queue
                # before its `at` input is ready (head-of-line block) ----
                oav = oa[b].rearrange("(c p) w h -> p c w h", c=NCH)
                qn = 4 if b == B - 1 else 2
                qs = WT // qn

                def col_tail(w, at_w, v_sb_w):
                    # attn^T via PE transpose
                    atp = ptp.tile([128, 128], bf16, tag="atp")
                    nc.tensor.transpose(atp[:], at_w[:], id_sb[:])
                    ats = spool.tile([128, 128], bf16, tag=f"ats{w % 2}")
                    if w % 2 == 0:
                        nc.vector.tensor_copy(ats[:], atp[:])
                    else:
                        nc.scalar.activation(ats[:], atp[:], AF.Copy)

                    # AV: av(c, i) = V^T.T @ attn^T, 4 bf16 matmuls
                    av = pav.tile([128, 512], f32, tag="av")
                    for ci in range(NCH):
                        nc.tensor.matmul(av[:, ci * 128:(ci + 1) * 128],
                                         v_sb_w[:, ci * 128:(ci + 1) * 128],
                                         ats[:], start=True, stop=True)

                    # drain: out = av + x (residual)
                    av3 = av[:].rearrange("p (c h) -> p c h", c=NCH)
                    dst = os4[:, :, w, :]
                    res = xs4[:, :, w, :]
                    if bv_is_zero:
                        nc.vector.tensor_add(dst, av3, res)
                    else:
                        for ci in range(NCH):
                            nc.vector.scalar_tensor_tensor(
                                dst[:, ci], av3[:, ci],
                                gbv_sb[:, ci:ci + 1],
                                res[:, ci], ALU.add, ALU.add)

                    # store slab pieces as soon as their drains are done;
                    # finer pieces for the last batch to shrink the tail
                    if (w + 1) % qs == 0 and w + 1 < WT:
                        lo = w + 1 - qs
                        nc.sync.dma_start(oav[:, :, lo:w + 1, :],
                                          os4[:, :, lo:w + 1, :])

                pending = None
                for w in range(WT):
                    # ---- scores: one bf16 matmul (i, j) ----
                    sc = psc.tile([128, 128], f32, tag="sc")
                    nc.tensor.matmul(sc[:], qk3[0:64, w, :], ks3[:, w, :],
                                     start=True, stop=True)

                    # ---- V^T: fp8 DoubleRow (K=256/mm) ----
                    vt = pvt.tile([128, 512], f32, tag="vt")
                    for cp in (0, 2):
                        nc.tensor.matmul(vt[:], xf84[:, cp:cp + 2, :, w],
                                         wv84[:, cp:cp + 2, :],
                                         perf_mode=DR,
                                         start=(cp == 0), stop=(cp == 2))
                    v_sb = spool.tile([128, 512], bf16, tag=f"v{w % 2}")
                    if w % 2 == 0:
                        nc.scalar.activation(v_sb[:], vt[:], AF.Copy,
                                             scale=1.0 / FP8_WSCALE)
                    else:
                        nc.vector.tensor_scalar_mul(v_sb[:], vt[:],
                                                    1.0 / FP8_WSCALE)

                    # ---- softmax head ----
                    ex = spool.tile([128, 128], bf16, tag=f"ex{w % 2}")
                    sums = spool.tile([128, 1], f32, tag=f"sums{w % 2}")
                    nc.scalar.activation(ex[:], sc[:], AF.Exp,
                                         accum_out=sums[:])
                    rr = spool.tile([128, 1], f32, tag=f"rr{w % 2}")
                    nc.vector.reciprocal_approx_fast(rr[:], sums[:])
                    at = spool.tile([128, 128], bf16, tag=f"at{w % 2}")
                    nc.vector.tensor_scalar_mul(at[:], ex[:], rr[:])

                    # ---- previous column's transpose/AV/drain ----
                    if pending is not None:
                        col_tail(*pending)
                    pending = (w, at, v_sb)
                col_tail(*pending)

                # ---- final store piece ----
                nc.sync.dma_start(oav[:, :, WT - qs:WT, :],
                                  os4[:, :, WT - qs:WT, :])

    nc.compile()
    return nc


def kernel(x, Wq, bq, Wk, bk, Wv, bv, gamma):
    global LAST_RESULTS
    x = np.asarray(x, dtype=np.float32)
    Wq = np.asarray(Wq, dtype=np.float32)
    bq = np.asarray(bq, dtype=np.float32)
    Wk = np.asarray(Wk, dtype=np.float32)
    bk = np.asarray(bk, dtype=np.float32)
    Wv = np.asarray(Wv, dtype=np.float32)
    bv = np.asarray(bv, dtype=np.float32)
    g = float(np.asarray(gamma, dtype=np.float32).reshape(-1)[0])

    bv_is_zero = not np.any(bv)
    bqk_is_zero = not (np.any(bq) or np.any(bk))
    nc = _build(bv_is_zero, bqk_is_zero)

    wqkT = np.ascontiguousarray(
        np.concatenate([Wq, Wk], axis=0).T).astype(ml_dtypes.bfloat16)
    bqk = np.concatenate([bq, bk], axis=0).reshape(128, 1)
    # V weights in fp8e4m3, pre-scaled out of the subnormal range; the
    # on-device PSUM->SBUF copy divides the scale back out
    wvT = np.ascontiguousarray((FP8_WSCALE * g * Wv).T).astype(
        mybir.dt.np(f8))                                                 # (C, C)
    gbv = np.ascontiguousarray((g * bv).reshape(NCH, 128).T)             # (128, NCH)

    in_maps = []
    for k in range(N_CORES):
        xsl = x[:, :, :, k * WT:(k + 1) * WT]                # (B, C, H, WT)
        xw = np.ascontiguousarray(xsl.transpose(0, 1, 3, 2))  # (B, C, WT, H)
        in_maps.append({
            "xb": xw.astype(ml_dtypes.bfloat16),
            "x8": np.ascontiguousarray(xsl).astype(mybir.dt.np(f8)),
            "wqkT": wqkT,
            "bqk": bqk,
            "wvT": wvT,
            "gbv": gbv,
        })

    res = run_bass_kernel_spmd(nc, in_maps, core_ids=list(range(N_CORES)),
                               trace=TRACE)
    LAST_RESULTS = res

    out = np.empty((B, C, H, W), dtype=np.float32)
    for k in range(N_CORES):
        # device output is (B, C, WT, H) w-major; transpose back
        out[:, :, :, k * WT:(k + 1) * WT] = \
            res.results[k]["out"].transpose(0, 1, 3, 2).astype(np.float32)
    return out
